# revision 31
# baseline (speedup 1.0000x reference)
"""MultiLinearUpsampling Trainium2 kernel.

Problem: out[b, t, :] = W[lidx[t]] @ pooled[b, segc[t], :]  (zero for invalid t)
where segc/lidx derive from sorted pooling_indices (ragged segments).

Strategy (v2: output-dim split)
-------------------------------
Only sum_l N_l matvecs are unique per batch (N_l = #segments with
len > l).  Sorting segments by length (desc) makes each linear l's
column set a PREFIX of one flat (rank, batch) column axis: linear l
applies to flat columns [0, 8*N_l).

Each of the 8 cores owns a 128-row slice of D_out and computes ALL
columns for ALL 16 linears on that slice: perfectly balanced, no
packing waste, and X is a single shared gather.  Per core per l:
Y_l[m, c] = sum_d W[l, m_slice, d] * X[d, c] for c < 8*N_l, computed
as 8 contraction tiles x <=512-wide PSUM windows, fp16 in / fp32
accumulate / fp16 out.  l runs smallest-prefix first so early X
chunks enable compute immediately and outputs drain throughout.

The host scatters Y columns to their t positions (including the
l = L-1 tail replication) and zero-fills invalid t.
"""

from contextlib import ExitStack

import numpy as np

import concourse.bass as bass  # noqa: F401  (bass types via bacc)
import concourse.mybir as mybir
import concourse.tile as tile
from concourse import bacc
from concourse.bass_utils import run_bass_kernel_spmd

F32 = mybir.dt.float32
F16 = mybir.dt.float16

B = 8          # batch
N = 512        # segments
D = 1024       # D_in == D_out
L = 16         # linears
NCORES = 8
KC = 8         # contraction chunks of 128
MSLICE = 128   # out-dim rows per core
WMAX = 512     # PSUM tile width (one bank of fp32); matmul windows are
               # capped at 506 -- 512-wide matmuls throttle the PE clock


def _bounds(F_ls):
    """X region boundaries [0, .., F]: regions are DMA'd whole, in order,
    so each boundary is a prefix the early schedule can run on.  The
    first region is exactly the first processed linear's prefix so that
    linear starts as early as possible."""
    srt = sorted(F_ls)
    F = srt[-1]
    cand = {F}
    for i in (3, 11):
        if len(srt) > i + 1:
            cand.add(srt[i])
    return [0] + sorted(c for c in cand if c > 0)


def _windows(F_l, bounds):
    """Split [0, F_l) at region bounds, even-split into <=506 windows
    (512 exactly throttles the PE clock).  Returns (region, start, len)
    with start relative to region start."""
    out = []
    for ri in range(len(bounds) - 1):
        lo, hi = bounds[ri], min(bounds[ri + 1], F_l)
        if hi <= lo:
            break
        width = hi - lo
        nw = -(-width // 506)
        base, rem = divmod(width, nw)
        off = 0
        for j in range(nw):
            ln = base + (1 if j < rem else 0)
            out.append((ri, off, ln))
            off += ln
    return out


def _build_program(F_ls):
    """F_ls: per-linear flat-column prefix lengths (ascending process
    order except a small final block, zeros removed)."""
    nc = bacc.Bacc("TRN2", target_bir_lowering=False, debug=False)
    bounds = _bounds(F_ls)
    nreg = len(bounds) - 1
    widths = [bounds[i + 1] - bounds[i] for i in range(nreg)]
    NL = len(F_ls)
    total_cols = sum(F_ls)

    xs = [
        nc.dram_tensor(f"x{r}", (KC, 128, widths[r]), F16, kind="ExternalInput")
        for r in range(nreg)
    ]
    wh = nc.dram_tensor("w", (NL, 128, KC * 128), F16, kind="ExternalInput")
    y = nc.dram_tensor("y", (128, total_cols), F16, kind="ExternalOutput")

    with tile.TileContext(nc) as tc, ExitStack() as ctx:
        xpool = ctx.enter_context(tc.tile_pool(name="x", bufs=1))
        wpool = ctx.enter_context(tc.tile_pool(name="w", bufs=1))
        ypool = ctx.enter_context(tc.tile_pool(name="y", bufs=1))
        ppool = ctx.enter_context(tc.tile_pool(name="ps", bufs=8, space="PSUM"))

        # PE clock warm-up: any PE idle gap >~1us drops the clock back to
        # 0.65GHz with a multi-us re-ramp, so keep the array busy on
        # scratch matmuls from kernel start until slightly PAST the point
        # where the first region + first weights have landed -- the ramp
        # credit then carries into real work with no cold restart.
        sx = xpool.tile([128, 256], F16, tag="scratch_x", name="sx")
        sw = xpool.tile([128, 128], F16, tag="scratch_w", name="sw")
        nc.vector.memset(sx[:], 0.0)
        nc.vector.memset(sw[:], 0.0)
        spt = ppool.tile([128, WMAX], F32, tag="ps", name="spt")
        for _ in range(20):
            nc.tensor.matmul(spt[:, :256], sw[:], sx[:], start=True, stop=True)

        # Transfers serialize per DMA ring and the two HWDGE rings share
        # the 16 SDMA engines, so order transfers globally by when the
        # compute schedule needs them: X regions in column order plus the
        # first linear's W on the sync ring; the next few W planes race
        # on the scalar ring; late W planes queue behind all X on sync.
        NW_EARLY = min(7, NL)
        xt = {}
        for r in range(nreg):
            xt[r] = xpool.tile([128, KC, widths[r]], F16, tag=f"x{r}", name=f"x{r}")
        xa0 = xpool.tile([128, widths[0]], F16, tag="xa0", name="xa0")
        wt = {}
        for li in range(NL):
            wt[li] = wpool.tile([128, KC * 128], F16, tag=f"w{li}", name=f"w{li}")
        src0 = xs[0].ap().rearrange("kc kp w -> kp kc w")
        nc.sync.dma_start(xa0[:], src0[:, 0])
        nc.sync.dma_start(xt[0][:, 1:], src0[:, 1:])
        for r in range(1, nreg):
            nc.sync.dma_start(
                xt[r][:], xs[r].ap().rearrange("kc kp w -> kp kc w")
            )
        for li in range(NL):
            if li < NW_EARLY:
                nc.scalar.dma_start(wt[li][:], wh.ap()[li])
            else:
                nc.sync.dma_start(wt[li][:], wh.ap()[li])

        def moving(ri, k, ws, wl):
            if ri == 0 and k == 0:
                return xa0[:, ws : ws + wl]
            return xt[ri][:, k, ws : ws + wl]

        # column offsets of each linear's Y block in the output tensor
        offs = []
        off = 0
        for li in range(NL):
            offs.append(off)
            off += F_ls[li]

        # Region-major schedule: all linears' region-r windows run before
        # any region-r+1 window.  Region A alone (plus the streaming W
        # planes) supplies tens of us of compute, so every later X region
        # lands long before it is needed and the PE never starves after
        # the first couple of linears.  The last region runs largest-
        # first so the final Y drain is the smallest block.
        yts = {}
        ydrained = {li: 0 for li in range(NL)}
        winsl = {li: _windows(F_ls[li], bounds) for li in range(NL)}
        for r in range(nreg):
            order = [li for li in range(NL) if F_ls[li] > bounds[r]]
            if r == nreg - 1 and len(order) > 1:
                order = order[::-1]
            for li in order:
                wins = [w for w in winsl[li] if w[0] == r]
                if li not in yts:
                    yts[li] = ypool.tile(
                        [128, F_ls[li]], F16, tag=f"yt{li}", name=f"yt{li}"
                    )
                yt = yts[li]
                for g0 in range(0, len(wins), 4):
                    grp = wins[g0 : g0 + 4]
                    pts = [
                        ppool.tile(
                            [128, WMAX], F32, tag="ps", name=f"ps{r}_{li}_{g0}_{j}"
                        )
                        for j in range(len(grp))
                    ]
                    for k in range(KC):
                        for (ri, ws, wl), pt in zip(grp, pts):
                            nc.tensor.matmul(
                                pt[:, :wl],
                                wt[li][:, k * 128 : (k + 1) * 128],
                                moving(ri, k, ws, wl),
                                start=(k == 0),
                                stop=(k == KC - 1),
                            )
                    for (ri, ws, wl), pt in zip(grp, pts):
                        woff = bounds[ri] + ws
                        nc.vector.tensor_copy(yt[:, woff : woff + wl], pt[:, :wl])
                # drain this linear's completed span; finish on its last
                # region.  The final block of the program goes on the
                # (idle by then) HWDGE sync ring.
                done = min(bounds[r + 1], F_ls[li])
                last = done >= F_ls[li]
                dr = ydrained[li]
                if last or done - dr >= 688:
                    final = last and r == nreg - 1 and li == order[-1]
                    eng = nc.sync if final else nc.gpsimd
                    eng.dma_start(
                        y.ap()[:, offs[li] + dr : offs[li] + done],
                        yt[:, dr:done],
                    )
                    ydrained[li] = done

    nc.compile()
    return nc


# ---------------------------------------------------------------------------
# host wrapper
# ---------------------------------------------------------------------------

def _segment_structure(idx, T):
    t = np.arange(T)
    seg = np.searchsorted(idx, t, side="left")
    valid = seg < N
    segc = np.clip(seg, 0, N - 1)
    start = np.where(segc > 0, idx[np.maximum(segc - 1, 0)] + 1, 0)
    lidx = np.minimum(t - start, L - 1).astype(np.int64)
    lens = np.bincount(segc[valid], minlength=N)
    return t, seg, valid, segc, lidx, lens


def _install_ntff_hook():
    """Profiling-only: register the axon NTFF profile hook (dev use)."""
    import sys
    import types

    try:
        import antenv

        if "antenv.axon_hooks" not in sys.modules:
            mod = types.ModuleType("antenv.axon_hooks")
            holder = [None]
            mod.set_axon_ntff_profile_hook = lambda h: holder.__setitem__(0, h)
            mod.get_axon_ntff_profile_hook = lambda: holder[0]
            sys.modules["antenv.axon_hooks"] = mod
            antenv.axon_hooks = mod
            from trn_agent_boot.trn_boot import _ntff_profile_via_ctypes

            mod.set_axon_ntff_profile_hook(
                _ntff_profile_via_ctypes("/opt/axon/libaxon_pjrt.so")
            )
    except Exception as e:
        print(f"NTFF hook install failed: {e}")


def kernel(pooled_vectors, W, pooling_indices, target_length, _trace=False):
    pooled = np.asarray(pooled_vectors, dtype=np.float32)
    Wf = np.asarray(W, dtype=np.float32)
    idx = np.asarray(pooling_indices).astype(np.int64)
    T = int(np.asarray(target_length))

    t, seg, valid, segc, lidx, lens = _segment_structure(idx, T)

    order = np.argsort(-lens, kind="stable")      # segments by len desc
    rank_of_seg = np.empty(N, dtype=np.int64)
    rank_of_seg[order] = np.arange(N)
    N_l = (lens[None, :] > np.arange(L)[:, None]).sum(axis=1)  # (L,)

    # process order: ascending prefix length (early compute only needs
    # early X chunks), except the smallest linear moves to the end so
    # the final output drain is tiny; zero-size linears skipped
    proc = [l for l in np.argsort(N_l, kind="stable") if N_l[l] > 0]
    F_ls = [8 * int(N_l[l]) for l in proc]
    F = max(F_ls) if F_ls else 8

    nc = _build_program(F_ls)

    # flat column axis: (rank-major, batch-minor), ranks with len>0 only
    n0 = F // 8
    Xh = (
        pooled.transpose(2, 1, 0)[:, order[:n0], :]
        .reshape(D, F)
        .astype(np.float16)
    )  # (D, F), col = r*8 + b
    bounds = _bounds(F_ls)
    xregions = [
        np.ascontiguousarray(
            Xh[:, bounds[r] : bounds[r + 1]].reshape(KC, 128, -1)
        )
        for r in range(len(bounds) - 1)
    ]

    in_maps = []
    for c in range(NCORES):
        # W[l, m_slice, d] -> (l, kp, kc*128+m) with d = kc*128 + kp
        wc = (
            Wf[np.array(proc), c * 128 : (c + 1) * 128, :]
            .transpose(0, 2, 1)
            .reshape(len(proc), KC, 128, 128)
            .transpose(0, 2, 1, 3)
            .reshape(len(proc), 128, KC * 128)
            .astype(np.float16)
        )
        im = {"w": np.ascontiguousarray(wc)}
        for r, xr in enumerate(xregions):
            im[f"x{r}"] = xr
        in_maps.append(im)

    kwargs = {}
    if _trace:
        _install_ntff_hook()
        kwargs = dict(trace=True)
    res = run_bass_kernel_spmd(nc, in_maps, core_ids=list(range(NCORES)), **kwargs)
    results = res.results

    # assemble (D, total_cols) then scatter to (B, T, D)
    Yall = np.concatenate(
        [np.asarray(results[c]["y"]) for c in range(NCORES)], axis=0
    )  # (1024, total_cols) f16
    col_off = np.zeros(L, dtype=np.int64)
    off = 0
    for li, l in enumerate(proc):
        col_off[l] = off
        off += F_ls[li]

    Dout = Wf.shape[1]
    out = np.zeros((B, T, Dout), dtype=np.float32)
    tv = t[valid]
    ci = col_off[lidx[tv]] + rank_of_seg[segc[tv]] * 8  # (Tv,)
    cib = ci[:, None] + np.arange(B)[None, :]           # (Tv, B)
    out[:, tv, :] = Yall[:, cib].transpose(2, 1, 0).astype(np.float32)

    if _trace:
        kernel._last_exec_time_ns = res.exec_time_ns
        kernel._last_results = res
    return out


# revision 32
# speedup vs baseline: 1.1095x; 1.1095x over previous
"""MultiLinearUpsampling Trainium2 kernel.

Problem: out[b, t, :] = W[lidx[t]] @ pooled[b, segc[t], :]  (zero for invalid t)
where segc/lidx derive from sorted pooling_indices (ragged segments).

Strategy: output-dim split, region-major schedule
-------------------------------------------------
Only sum_l N_l matvecs are unique per batch (N_l = #segments with
len > l).  Sorting segments by length (desc) makes each linear l's
column set a PREFIX of one flat (rank, batch) column axis: linear l
applies to flat columns [0, 8*N_l).

Each of the 8 cores owns a 128-row slice of D_out and computes ALL
columns for ALL 16 linears on that slice: perfectly balanced (exactly
sum_l 8*N_l = ~232k PE columns per core, no packing waste), and X is
one shared gather.  Per core per l: Y_l[m, c] = sum_d W[l, m_slice, d]
* X[d, c] for c < 8*N_l, as 8 contraction tiles x <=506-wide PSUM
windows (512 exactly throttles the PE clock), fp16 in / fp32
accumulate / fp16 out.

The schedule is REGION-major: X is split into a few column regions,
and every linear's region-r windows run before any region-r+1 window.
Region A alone feeds tens of us of compute (every linear reuses its
columns), so later X regions and weight planes stream in behind
compute on the two HWDGE DMA rings (sync: X + late W; scalar: early
W) and the PE never starves after the first couple of linears.
Scratch warm-up matmuls bridge the initial DMA wait so the PE clock
(which resets to 0.65GHz on any >~1us idle gap) ramps once.  The last
region runs largest-linear first so the final output drain is tiny;
Y blocks drain incrementally on the gpsimd SWDGE ring.

The host scatters Y columns to their t positions (including the
l = L-1 tail replication) and zero-fills invalid t.
"""

from contextlib import ExitStack

import numpy as np

import concourse.bass as bass  # noqa: F401  (bass types via bacc)
import concourse.mybir as mybir
import concourse.tile as tile
from concourse import bacc
from concourse.bass_utils import run_bass_kernel_spmd

F32 = mybir.dt.float32
F16 = mybir.dt.float16

B = 8          # batch
N = 512        # segments
D = 1024       # D_in == D_out
L = 16         # linears
NCORES = 8
KC = 8         # contraction chunks of 128
MSLICE = 128   # out-dim rows per core
WMAX = 512     # PSUM tile width (one bank of fp32); matmul windows are
               # capped at 506 -- 512-wide matmuls throttle the PE clock


def _bounds(F_ls):
    """X region boundaries [0, .., F]: regions are DMA'd whole, in order,
    so each boundary is a prefix the early schedule can run on.  The
    first region is exactly the first processed linear's prefix so that
    linear starts as early as possible."""
    srt = sorted(F_ls)
    F = srt[-1]
    cand = {F}
    for i in (3, 11):
        if len(srt) > i + 1:
            cand.add(srt[i])
    return [0] + sorted(c for c in cand if c > 0)


def _windows(F_l, bounds):
    """Split [0, F_l) at region bounds, even-split into <=506 windows
    (512 exactly throttles the PE clock).  Returns (region, start, len)
    with start relative to region start."""
    out = []
    for ri in range(len(bounds) - 1):
        lo, hi = bounds[ri], min(bounds[ri + 1], F_l)
        if hi <= lo:
            break
        width = hi - lo
        nw = -(-width // 506)
        base, rem = divmod(width, nw)
        off = 0
        for j in range(nw):
            ln = base + (1 if j < rem else 0)
            out.append((ri, off, ln))
            off += ln
    return out


def _build_program(F_ls):
    """F_ls: per-linear flat-column prefix lengths (ascending process
    order except a small final block, zeros removed)."""
    nc = bacc.Bacc("TRN2", target_bir_lowering=False, debug=False)
    bounds = _bounds(F_ls)
    nreg = len(bounds) - 1
    widths = [bounds[i + 1] - bounds[i] for i in range(nreg)]
    NL = len(F_ls)
    total_cols = sum(F_ls)

    xs = [
        nc.dram_tensor(f"x{r}", (KC, 128, widths[r]), F16, kind="ExternalInput")
        for r in range(nreg)
    ]
    wh = nc.dram_tensor("w", (NL, 128, KC * 128), F16, kind="ExternalInput")
    y = nc.dram_tensor("y", (128, total_cols), F16, kind="ExternalOutput")

    with tile.TileContext(nc) as tc, ExitStack() as ctx:
        xpool = ctx.enter_context(tc.tile_pool(name="x", bufs=1))
        wpool = ctx.enter_context(tc.tile_pool(name="w", bufs=1))
        ypool = ctx.enter_context(tc.tile_pool(name="y", bufs=1))
        ppool = ctx.enter_context(tc.tile_pool(name="ps", bufs=8, space="PSUM"))

        # PE clock warm-up: any PE idle gap >~1us drops the clock back to
        # 0.65GHz with a multi-us re-ramp, so keep the array busy on
        # scratch matmuls from kernel start until slightly PAST the point
        # where the first region + first weights have landed -- the ramp
        # credit then carries into real work with no cold restart.
        sx = xpool.tile([128, 256], F16, tag="scratch_x", name="sx")
        sw = xpool.tile([128, 128], F16, tag="scratch_w", name="sw")
        nc.vector.memset(sx[:], 0.0)
        nc.vector.memset(sw[:], 0.0)
        spt = ppool.tile([128, WMAX], F32, tag="ps", name="spt")
        for _ in range(20):
            nc.tensor.matmul(spt[:, :256], sw[:], sx[:], start=True, stop=True)

        # Transfers serialize per DMA ring and the two HWDGE rings share
        # the 16 SDMA engines, so order transfers globally by when the
        # compute schedule needs them: X regions in column order plus the
        # first linear's W on the sync ring; the next few W planes race
        # on the scalar ring; late W planes queue behind all X on sync.
        NW_EARLY = min(7, NL)
        xt = {}
        for r in range(nreg):
            xt[r] = xpool.tile([128, KC, widths[r]], F16, tag=f"x{r}", name=f"x{r}")
        xa0 = xpool.tile([128, widths[0]], F16, tag="xa0", name="xa0")
        wt = {}
        for li in range(NL):
            wt[li] = wpool.tile([128, KC * 128], F16, tag=f"w{li}", name=f"w{li}")
        src0 = xs[0].ap().rearrange("kc kp w -> kp kc w")
        nc.sync.dma_start(xa0[:], src0[:, 0])
        nc.sync.dma_start(xt[0][:, 1:], src0[:, 1:])
        for r in range(1, nreg):
            nc.sync.dma_start(
                xt[r][:], xs[r].ap().rearrange("kc kp w -> kp kc w")
            )
        for li in range(NL):
            if li < NW_EARLY:
                nc.scalar.dma_start(wt[li][:], wh.ap()[li])
            else:
                nc.sync.dma_start(wt[li][:], wh.ap()[li])

        def moving(ri, k, ws, wl):
            if ri == 0 and k == 0:
                return xa0[:, ws : ws + wl]
            return xt[ri][:, k, ws : ws + wl]

        # column offsets of each linear's Y block in the output tensor
        offs = []
        off = 0
        for li in range(NL):
            offs.append(off)
            off += F_ls[li]

        # Region-major schedule: all linears' region-r windows run before
        # any region-r+1 window.  Region A alone (plus the streaming W
        # planes) supplies tens of us of compute, so every later X region
        # lands long before it is needed and the PE never starves after
        # the first couple of linears.  The last region runs largest-
        # first so the final Y drain is the smallest block.
        yts = {}
        ydrained = {li: 0 for li in range(NL)}
        winsl = {li: _windows(F_ls[li], bounds) for li in range(NL)}
        for r in range(nreg):
            order = [li for li in range(NL) if F_ls[li] > bounds[r]]
            if r == nreg - 1 and len(order) > 1:
                order = order[::-1]
            for li in order:
                wins = [w for w in winsl[li] if w[0] == r]
                if li not in yts:
                    yts[li] = ypool.tile(
                        [128, F_ls[li]], F16, tag=f"yt{li}", name=f"yt{li}"
                    )
                yt = yts[li]
                for g0 in range(0, len(wins), 4):
                    grp = wins[g0 : g0 + 4]
                    pts = [
                        ppool.tile(
                            [128, WMAX], F32, tag="ps", name=f"ps{r}_{li}_{g0}_{j}"
                        )
                        for j in range(len(grp))
                    ]
                    for k in range(KC):
                        for (ri, ws, wl), pt in zip(grp, pts):
                            nc.tensor.matmul(
                                pt[:, :wl],
                                wt[li][:, k * 128 : (k + 1) * 128],
                                moving(ri, k, ws, wl),
                                start=(k == 0),
                                stop=(k == KC - 1),
                            )
                    for (ri, ws, wl), pt in zip(grp, pts):
                        woff = bounds[ri] + ws
                        nc.vector.tensor_copy(yt[:, woff : woff + wl], pt[:, :wl])
                # drain this linear's completed span; finish on its last
                # region.  The final block of the program goes on the
                # (idle by then) HWDGE sync ring.
                done = min(bounds[r + 1], F_ls[li])
                last = done >= F_ls[li]
                dr = ydrained[li]
                if last or done - dr >= 688:
                    final = last and r == nreg - 1 and li == order[-1]
                    eng = nc.sync if final else nc.gpsimd
                    eng.dma_start(
                        y.ap()[:, offs[li] + dr : offs[li] + done],
                        yt[:, dr:done],
                    )
                    ydrained[li] = done

    nc.compile()
    return nc


# ---------------------------------------------------------------------------
# host wrapper
# ---------------------------------------------------------------------------

def _segment_structure(idx, T):
    t = np.arange(T)
    seg = np.searchsorted(idx, t, side="left")
    valid = seg < N
    segc = np.clip(seg, 0, N - 1)
    start = np.where(segc > 0, idx[np.maximum(segc - 1, 0)] + 1, 0)
    lidx = np.minimum(t - start, L - 1).astype(np.int64)
    lens = np.bincount(segc[valid], minlength=N)
    return t, seg, valid, segc, lidx, lens


def _install_ntff_hook():
    """Profiling-only: register the axon NTFF profile hook (dev use)."""
    import sys
    import types

    try:
        import antenv

        if "antenv.axon_hooks" not in sys.modules:
            mod = types.ModuleType("antenv.axon_hooks")
            holder = [None]
            mod.set_axon_ntff_profile_hook = lambda h: holder.__setitem__(0, h)
            mod.get_axon_ntff_profile_hook = lambda: holder[0]
            sys.modules["antenv.axon_hooks"] = mod
            antenv.axon_hooks = mod
            from trn_agent_boot.trn_boot import _ntff_profile_via_ctypes

            mod.set_axon_ntff_profile_hook(
                _ntff_profile_via_ctypes("/opt/axon/libaxon_pjrt.so")
            )
    except Exception as e:
        print(f"NTFF hook install failed: {e}")


def kernel(pooled_vectors, W, pooling_indices, target_length, _trace=False):
    pooled = np.asarray(pooled_vectors, dtype=np.float32)
    Wf = np.asarray(W, dtype=np.float32)
    idx = np.asarray(pooling_indices).astype(np.int64)
    T = int(np.asarray(target_length))

    t, seg, valid, segc, lidx, lens = _segment_structure(idx, T)

    order = np.argsort(-lens, kind="stable")      # segments by len desc
    rank_of_seg = np.empty(N, dtype=np.int64)
    rank_of_seg[order] = np.arange(N)
    N_l = (lens[None, :] > np.arange(L)[:, None]).sum(axis=1)  # (L,)

    # process order: ascending prefix length (early compute only needs
    # early X chunks), except the smallest linear moves to the end so
    # the final output drain is tiny; zero-size linears skipped
    proc = [l for l in np.argsort(N_l, kind="stable") if N_l[l] > 0]
    F_ls = [8 * int(N_l[l]) for l in proc]
    F = max(F_ls) if F_ls else 8

    nc = _build_program(F_ls)

    # flat column axis: (rank-major, batch-minor), ranks with len>0 only
    n0 = F // 8
    Xh = (
        pooled.transpose(2, 1, 0)[:, order[:n0], :]
        .reshape(D, F)
        .astype(np.float16)
    )  # (D, F), col = r*8 + b
    bounds = _bounds(F_ls)
    xregions = [
        np.ascontiguousarray(
            Xh[:, bounds[r] : bounds[r + 1]].reshape(KC, 128, -1)
        )
        for r in range(len(bounds) - 1)
    ]

    in_maps = []
    for c in range(NCORES):
        # W[l, m_slice, d] -> (l, kp, kc*128+m) with d = kc*128 + kp
        wc = (
            Wf[np.array(proc), c * 128 : (c + 1) * 128, :]
            .transpose(0, 2, 1)
            .reshape(len(proc), KC, 128, 128)
            .transpose(0, 2, 1, 3)
            .reshape(len(proc), 128, KC * 128)
            .astype(np.float16)
        )
        im = {"w": np.ascontiguousarray(wc)}
        for r, xr in enumerate(xregions):
            im[f"x{r}"] = xr
        in_maps.append(im)

    kwargs = {}
    if _trace:
        _install_ntff_hook()
        kwargs = dict(trace=True)
    res = run_bass_kernel_spmd(nc, in_maps, core_ids=list(range(NCORES)), **kwargs)
    results = res.results

    # assemble (D, total_cols) then scatter to (B, T, D)
    Yall = np.concatenate(
        [np.asarray(results[c]["y"]) for c in range(NCORES)], axis=0
    )  # (1024, total_cols) f16
    col_off = np.zeros(L, dtype=np.int64)
    off = 0
    for li, l in enumerate(proc):
        col_off[l] = off
        off += F_ls[li]

    Dout = Wf.shape[1]
    out = np.zeros((B, T, Dout), dtype=np.float32)
    tv = t[valid]
    ci = col_off[lidx[tv]] + rank_of_seg[segc[tv]] * 8  # (Tv,)
    cib = ci[:, None] + np.arange(B)[None, :]           # (Tv, B)
    out[:, tv, :] = Yall[:, cib].transpose(2, 1, 0).astype(np.float32)

    if _trace:
        kernel._last_exec_time_ns = res.exec_time_ns
        kernel._last_results = res
    return out


# revision 33
# speedup vs baseline: 1.1831x; 1.0663x over previous
"""MultiLinearUpsampling Trainium2 kernel.

Problem: out[b, t, :] = W[lidx[t]] @ pooled[b, segc[t], :]  (zero for invalid t)
where segc/lidx derive from sorted pooling_indices (ragged segments).

Strategy: output-dim split, region-major schedule
-------------------------------------------------
Only sum_l N_l matvecs are unique per batch (N_l = #segments with
len > l).  Sorting segments by length (desc) makes each linear l's
column set a PREFIX of one flat (rank, batch) column axis: linear l
applies to flat columns [0, 8*N_l).

Each of the 8 cores owns a 128-row slice of D_out and computes ALL
columns for ALL 16 linears on that slice: perfectly balanced (exactly
sum_l 8*N_l = ~232k PE columns per core, no packing waste), and X is
one shared gather.  Per core per l: Y_l[m, c] = sum_d W[l, m_slice, d]
* X[d, c] for c < 8*N_l, as 8 contraction tiles x <=506-wide PSUM
windows (512 exactly throttles the PE clock), fp16 in / fp32
accumulate / fp16 out.

The schedule is REGION-major: X is split into a few column regions,
and every linear's region-r windows run before any region-r+1 window.
Region A alone feeds tens of us of compute (every linear reuses its
columns), so later X regions and weight planes stream in behind
compute on the two HWDGE DMA rings (sync: X + late W; scalar: early
W) and the PE never starves after the first couple of linears.
Scratch warm-up matmuls bridge the initial DMA wait so the PE clock
(which resets to 0.65GHz on any >~1us idle gap) ramps once.  The last
region runs largest-linear first so the final output drain is tiny;
Y blocks drain incrementally on the gpsimd SWDGE ring.

The host scatters Y columns to their t positions (including the
l = L-1 tail replication) and zero-fills invalid t.
"""

from contextlib import ExitStack

import numpy as np

import concourse.bass as bass  # noqa: F401  (bass types via bacc)
import concourse.mybir as mybir
import concourse.tile as tile
from concourse import bacc
from concourse.bass_utils import run_bass_kernel_spmd

F32 = mybir.dt.float32
F16 = mybir.dt.float16

B = 8          # batch
N = 512        # segments
D = 1024       # D_in == D_out
L = 16         # linears
NCORES = 8
KC = 8         # contraction chunks of 128
MSLICE = 128   # out-dim rows per core
WMAX = 512     # PSUM tile width (one bank of fp32); matmul windows are
               # capped at 506 -- 512-wide matmuls throttle the PE clock


def _bounds(F_ls):
    """X region boundaries [0, .., F]: regions are DMA'd whole, in order,
    so each boundary is a prefix the early schedule can run on.  The
    first region is exactly the first processed linear's prefix so that
    linear starts as early as possible."""
    srt = sorted(F_ls)
    F = srt[-1]
    cand = {F}
    for i in (3, 11):
        if len(srt) > i + 1:
            cand.add(srt[i])
    return [0] + sorted(c for c in cand if c > 0)


def _windows(F_l, bounds):
    """Split [0, F_l) at region bounds, even-split into <=506 windows
    (512 exactly throttles the PE clock).  Returns (region, start, len)
    with start relative to region start."""
    out = []
    for ri in range(len(bounds) - 1):
        lo, hi = bounds[ri], min(bounds[ri + 1], F_l)
        if hi <= lo:
            break
        width = hi - lo
        nw = -(-width // 506)
        base, rem = divmod(width, nw)
        off = 0
        for j in range(nw):
            ln = base + (1 if j < rem else 0)
            out.append((ri, off, ln))
            off += ln
    return out


def _build_program(F_ls):
    """F_ls: per-linear flat-column prefix lengths (ascending process
    order except a small final block, zeros removed)."""
    nc = bacc.Bacc("TRN2", target_bir_lowering=False, debug=False)
    bounds = _bounds(F_ls)
    nreg = len(bounds) - 1
    widths = [bounds[i + 1] - bounds[i] for i in range(nreg)]
    NL = len(F_ls)
    total_cols = sum(F_ls)

    xs = [
        nc.dram_tensor(f"x{r}", (KC, 128, widths[r]), F16, kind="ExternalInput")
        for r in range(nreg)
    ]
    wh = nc.dram_tensor("w", (NL, 128, KC * 128), F16, kind="ExternalInput")
    y = nc.dram_tensor("y", (128, total_cols), F16, kind="ExternalOutput")

    with tile.TileContext(nc) as tc, ExitStack() as ctx:
        xpool = ctx.enter_context(tc.tile_pool(name="x", bufs=1))
        wpool = ctx.enter_context(tc.tile_pool(name="w", bufs=1))
        ypool = ctx.enter_context(tc.tile_pool(name="y", bufs=1))
        ppool = ctx.enter_context(tc.tile_pool(name="ps", bufs=8, space="PSUM"))

        # PE clock warm-up: any PE idle gap >~1us drops the clock back to
        # 0.65GHz with a multi-us re-ramp, so keep the array busy on
        # scratch matmuls from kernel start until slightly PAST the point
        # where the first region + first weights have landed -- the ramp
        # credit then carries into real work with no cold restart.
        sx = xpool.tile([128, 256], F16, tag="scratch_x", name="sx")
        sw = xpool.tile([128, 128], F16, tag="scratch_w", name="sw")
        nc.vector.memset(sx[:], 0.0)
        nc.vector.memset(sw[:], 0.0)
        spt = ppool.tile([128, WMAX], F32, tag="ps", name="spt")
        for _ in range(20):
            nc.tensor.matmul(spt[:, :256], sw[:], sx[:], start=True, stop=True)

        # Transfers serialize per DMA ring and the two HWDGE rings share
        # the 16 SDMA engines, so order transfers globally by when the
        # compute schedule needs them: X regions in column order plus the
        # first linear's W on the sync ring; the next few W planes race
        # on the scalar ring; late W planes queue behind all X on sync.
        NW_EARLY = min(7, NL)
        xt = {}
        for r in range(nreg):
            xt[r] = xpool.tile([128, KC, widths[r]], F16, tag=f"x{r}", name=f"x{r}")
        xa0 = xpool.tile([128, widths[0]], F16, tag="xa0", name="xa0")
        wt = {}
        for li in range(NL):
            wt[li] = wpool.tile([128, KC * 128], F16, tag=f"w{li}", name=f"w{li}")
        src0 = xs[0].ap().rearrange("kc kp w -> kp kc w")
        nc.sync.dma_start(xa0[:], src0[:, 0])
        nc.scalar.dma_start(wt[0][:], wh.ap()[0])
        nc.sync.dma_start(xt[0][:, 1:5], src0[:, 1:5])
        nc.scalar.dma_start(xt[0][:, 5:], src0[:, 5:])
        for r in range(1, nreg):
            nc.sync.dma_start(
                xt[r][:], xs[r].ap().rearrange("kc kp w -> kp kc w")
            )
        for li in range(1, NL):
            if li < NW_EARLY:
                nc.scalar.dma_start(wt[li][:], wh.ap()[li])
            else:
                nc.sync.dma_start(wt[li][:], wh.ap()[li])

        def moving(ri, k, ws, wl):
            if ri == 0 and k == 0:
                return xa0[:, ws : ws + wl]
            return xt[ri][:, k, ws : ws + wl]

        # column offsets of each linear's Y block in the output tensor
        offs = []
        off = 0
        for li in range(NL):
            offs.append(off)
            off += F_ls[li]

        # Region-major schedule: all linears' region-r windows run before
        # any region-r+1 window.  Region A alone (plus the streaming W
        # planes) supplies tens of us of compute, so every later X region
        # lands long before it is needed and the PE never starves after
        # the first couple of linears.  The last region runs largest-
        # first so the final Y drain is the smallest block.
        yts = {}
        ydrained = {li: 0 for li in range(NL)}
        winsl = {li: _windows(F_ls[li], bounds) for li in range(NL)}
        for r in range(nreg):
            order = [li for li in range(NL) if F_ls[li] > bounds[r]]
            if r == nreg - 1 and len(order) > 1:
                order = order[::-1]
            for li in order:
                wins = [w for w in winsl[li] if w[0] == r]
                if li not in yts:
                    yts[li] = ypool.tile(
                        [128, F_ls[li]], F16, tag=f"yt{li}", name=f"yt{li}"
                    )
                yt = yts[li]
                for g0 in range(0, len(wins), 4):
                    grp = wins[g0 : g0 + 4]
                    pts = [
                        ppool.tile(
                            [128, WMAX], F32, tag="ps", name=f"ps{r}_{li}_{g0}_{j}"
                        )
                        for j in range(len(grp))
                    ]
                    for k in range(KC):
                        for (ri, ws, wl), pt in zip(grp, pts):
                            nc.tensor.matmul(
                                pt[:, :wl],
                                wt[li][:, k * 128 : (k + 1) * 128],
                                moving(ri, k, ws, wl),
                                start=(k == 0),
                                stop=(k == KC - 1),
                            )
                    for (ri, ws, wl), pt in zip(grp, pts):
                        woff = bounds[ri] + ws
                        nc.vector.tensor_copy(yt[:, woff : woff + wl], pt[:, :wl])
                # drain this linear's completed span; finish on its last
                # region.  The final block of the program goes on the
                # (idle by then) HWDGE sync ring.
                done = min(bounds[r + 1], F_ls[li])
                last = done >= F_ls[li]
                dr = ydrained[li]
                if last or done - dr >= 688:
                    eng = nc.sync if r == nreg - 1 else nc.gpsimd
                    eng.dma_start(
                        y.ap()[:, offs[li] + dr : offs[li] + done],
                        yt[:, dr:done],
                    )
                    ydrained[li] = done

    nc.compile()
    return nc


# ---------------------------------------------------------------------------
# host wrapper
# ---------------------------------------------------------------------------

def _segment_structure(idx, T):
    t = np.arange(T)
    seg = np.searchsorted(idx, t, side="left")
    valid = seg < N
    segc = np.clip(seg, 0, N - 1)
    start = np.where(segc > 0, idx[np.maximum(segc - 1, 0)] + 1, 0)
    lidx = np.minimum(t - start, L - 1).astype(np.int64)
    lens = np.bincount(segc[valid], minlength=N)
    return t, seg, valid, segc, lidx, lens


def _install_ntff_hook():
    """Profiling-only: register the axon NTFF profile hook (dev use)."""
    import sys
    import types

    try:
        import antenv

        if "antenv.axon_hooks" not in sys.modules:
            mod = types.ModuleType("antenv.axon_hooks")
            holder = [None]
            mod.set_axon_ntff_profile_hook = lambda h: holder.__setitem__(0, h)
            mod.get_axon_ntff_profile_hook = lambda: holder[0]
            sys.modules["antenv.axon_hooks"] = mod
            antenv.axon_hooks = mod
            from trn_agent_boot.trn_boot import _ntff_profile_via_ctypes

            mod.set_axon_ntff_profile_hook(
                _ntff_profile_via_ctypes("/opt/axon/libaxon_pjrt.so")
            )
    except Exception as e:
        print(f"NTFF hook install failed: {e}")


def kernel(pooled_vectors, W, pooling_indices, target_length, _trace=False):
    pooled = np.asarray(pooled_vectors, dtype=np.float32)
    Wf = np.asarray(W, dtype=np.float32)
    idx = np.asarray(pooling_indices).astype(np.int64)
    T = int(np.asarray(target_length))

    t, seg, valid, segc, lidx, lens = _segment_structure(idx, T)

    order = np.argsort(-lens, kind="stable")      # segments by len desc
    rank_of_seg = np.empty(N, dtype=np.int64)
    rank_of_seg[order] = np.arange(N)
    N_l = (lens[None, :] > np.arange(L)[:, None]).sum(axis=1)  # (L,)

    # process order: ascending prefix length (early compute only needs
    # early X chunks), except the smallest linear moves to the end so
    # the final output drain is tiny; zero-size linears skipped
    proc = [l for l in np.argsort(N_l, kind="stable") if N_l[l] > 0]
    F_ls = [8 * int(N_l[l]) for l in proc]
    F = max(F_ls) if F_ls else 8

    nc = _build_program(F_ls)

    # flat column axis: (rank-major, batch-minor), ranks with len>0 only
    n0 = F // 8
    Xh = (
        pooled.transpose(2, 1, 0)[:, order[:n0], :]
        .reshape(D, F)
        .astype(np.float16)
    )  # (D, F), col = r*8 + b
    bounds = _bounds(F_ls)
    xregions = [
        np.ascontiguousarray(
            Xh[:, bounds[r] : bounds[r + 1]].reshape(KC, 128, -1)
        )
        for r in range(len(bounds) - 1)
    ]

    in_maps = []
    for c in range(NCORES):
        # W[l, m_slice, d] -> (l, kp, kc*128+m) with d = kc*128 + kp
        wc = (
            Wf[np.array(proc), c * 128 : (c + 1) * 128, :]
            .transpose(0, 2, 1)
            .reshape(len(proc), KC, 128, 128)
            .transpose(0, 2, 1, 3)
            .reshape(len(proc), 128, KC * 128)
            .astype(np.float16)
        )
        im = {"w": np.ascontiguousarray(wc)}
        for r, xr in enumerate(xregions):
            im[f"x{r}"] = xr
        in_maps.append(im)

    kwargs = {}
    if _trace:
        _install_ntff_hook()
        kwargs = dict(trace=True)
    res = run_bass_kernel_spmd(nc, in_maps, core_ids=list(range(NCORES)), **kwargs)
    results = res.results

    # assemble (D, total_cols) then scatter to (B, T, D)
    Yall = np.concatenate(
        [np.asarray(results[c]["y"]) for c in range(NCORES)], axis=0
    )  # (1024, total_cols) f16
    col_off = np.zeros(L, dtype=np.int64)
    off = 0
    for li, l in enumerate(proc):
        col_off[l] = off
        off += F_ls[li]

    Dout = Wf.shape[1]
    out = np.zeros((B, T, Dout), dtype=np.float32)
    tv = t[valid]
    ci = col_off[lidx[tv]] + rank_of_seg[segc[tv]] * 8  # (Tv,)
    cib = ci[:, None] + np.arange(B)[None, :]           # (Tv, B)
    out[:, tv, :] = Yall[:, cib].transpose(2, 1, 0).astype(np.float32)

    if _trace:
        kernel._last_exec_time_ns = res.exec_time_ns
        kernel._last_results = res
    return out
